# revision 5
# baseline (speedup 1.0000x reference)
"""Trainium2 Bass kernel for nn_AutoregressiveDecoder (gnn_message_passing).

Math (derived from the reference):
  With Ahat = max(adj, I), CS[i,u] = sum_{v<i} Ahat[v,u], deg_i = CS[i]^-1/2,
  row i of supp reduces to
    supp[i,u<i] = 0.5*tanh(deg_i(u) * (Ahat @ (deg_i^2 * relu(Yt_i)^T r_i))[u])
    supp[i,i]   = 0.5*tanh(q_i . q_i)
  where Yt_i = Z1^T (D_i Ahat), Z1 = z @ W1[:128],
    q_i = relu(Z1[i] + W1[128]) @ W2,  r_i = W2 @ q_i.
  Output = 0.5 z z^T + supp + supp^T.

v2 layout (vs baseline): deg is a pure function of adj, so ALL deg math is
host-precomputed: deg_j(v) AND deg_j(u)^2 are folded into the fp8 stage_a
operand (b3'), and the outer deg_i(u) ships as an f32 table (degf). This
deletes the on-device CS/rsqrt/transpose chain that serialized the old
preamble. W1's helper-row bias rides as an ACT per-partition bias column.
Inputs ship as 3 dram tensors in consumption order; PE filler matmuls keep
HAM activity up so the row loop runs warm.
"""

import numpy as np

N = 256
NCORES = 8
NPC = N // NCORES  # 32 rows per core

# (start_row, g) per group; mjg = 8*(start+g)
GROUPS = [(0, 8), (8, 4), (12, 4)] + [(j, 2) for j in range(16, 32, 2)]


def _s_offsets():
    offs = {}
    off = 0
    for gi, (j0, g) in enumerate(GROUPS):
        mjg = 8 * (j0 + g)
        nk = 1 if mjg <= 128 else 2
        offs[gi] = off
        off += nk * g * mjg
    return offs, off


S_OFFS, S_COLS = _s_offsets()  # S_COLS == 7808
BA_COLS = S_COLS + 512        # | b3' | ahbdr(2x256 DR) |

# bB (bf16) column map
ZT, W1A, W2H, W2T, ZOC, W1B = 0, 256, 512, 768, 1024, 1056
BB_COLS = 1058
# bC (bf16) column map: degf f32-bytes, one-hots, 32-identity
DEGF, OCB, IDT = 0, 128, 192
BC_COLS = 224

_PROGRAM = None
LAST_RESULTS = None
TRACE = False
TRACE_KW = {}


def _build_program():
    import concourse.bacc as bacc
    import concourse.mybir as mybir
    from concourse import tile

    F32 = mybir.dt.float32
    BF16 = mybir.dt.bfloat16
    FP8 = mybir.dt.float8e4
    AF = mybir.ActivationFunctionType
    ALU = mybir.AluOpType
    DR = mybir.MatmulPerfMode.DoubleRow

    nc = bacc.Bacc()

    bA_d = nc.dram_tensor("bA", [128, BA_COLS], FP8, kind="ExternalInput")
    bB_d = nc.dram_tensor("bB", [128, BB_COLS], BF16, kind="ExternalInput")
    bC_d = nc.dram_tensor("bC", [128, BC_COLS], BF16, kind="ExternalInput")
    out_d = nc.dram_tensor("outp", [128, 128], F32, kind="ExternalOutput")

    with tile.TileContext(nc) as tc, tc.tile_pool(name="persist", bufs=1) as P:
        bA = P.tile([128, BA_COLS], FP8, tag="bA", name="bA")
        bB = P.tile([128, BB_COLS], BF16, tag="bB", name="bB")
        bC = P.tile([128, BC_COLS], BF16, tag="bC", name="bC")
        # priority order: bB gates the preamble, bA chunks stream in
        # consumption order, bC is tail-only
        nc.sync.dma_start(bB[:], bB_d[:])
        nc.sync.dma_start(bA[:, 0:1408], bA_d[:, 0:1408])
        nc.sync.dma_start(bA[:, 1408:4224], bA_d[:, 1408:4224])
        nc.sync.dma_start(bA[:, 4224:BA_COLS], bA_d[:, 4224:BA_COLS])
        nc.sync.dma_start(bC[:], bC_d[:])

        ztb = bB[:, ZT:ZT + 256]
        w1ab = bB[:, W1A:W1A + 256]
        w2h = [bB[:, W2H:W2H + 128], bB[:, W2H + 128:W2H + 256]]
        w2tb = bB[:, W2T:W2T + 256]
        zoc = bB[:, ZOC:ZOC + 32]                   # 0.5-folded z cols
        w1b = bB[:, W1B:W1B + 2]                    # W1[128] as 2 bias cols
        ahbdr = bA[:, S_COLS:S_COLS + 512].rearrange("p (k c) -> p k c", k=2)
        degf = bC[:, DEGF:DEGF + 128].bitcast(F32)  # [128, 64]
        ocb = [bC[:, OCB:OCB + 32], bC[:, OCB + 32:OCB + 64]]
        ident32 = bC[0:32, IDT:IDT + 32]

        wsb = P.tile([128, 512], BF16, tag="wsb", name="wsb")
        nc.vector.memset(wsb[:], 0.0)
        onescol = P.tile([128, 1], BF16, tag="onescol", name="onescol")
        nc.gpsimd.memset(onescol[:], 1.0)
        rczdr = P.tile([128, 2, 1056], FP8, tag="rczdr", name="rczdr")
        nc.gpsimd.memset(rczdr[:], 0.0)

        # persistent SBUF intermediates
        z1dr = P.tile([128, 2, 256], FP8, tag="z1dr", name="z1dr")
        rbt = P.tile([128, 2, 256], BF16, tag="rbt", name="rbt")
        qtb = P.tile([128, 256], BF16, tag="qtb", name="qtb")
        sqb = P.tile([128, 256], BF16, tag="sqb", name="sqb")
        rsb = [P.tile([128, 256], BF16, tag=f"rsb{nb}", name=f"rsb{nb}")
               for nb in range(2)]
        rc_sb = P.tile([128, 2, 32], BF16, tag="rc_sb", name="rc_sb")
        tqh = P.tile([128, 2], F32, tag="tqh", name="tqh")
        dg = P.tile([128, 2, 32], F32, tag="dg", name="dg")
        spT = P.tile([32, 256], BF16, tag="spT", name="spT")
        spc = P.tile([128, 2, 32], FP8, tag="spc", name="spc")
        wt = P.tile([128, 64], F32, tag="wt", name="wt")
        tht = P.tile([128, 64], F32, tag="tht", name="tht")
        fin = P.tile([128, 2, 64], F32, tag="fin", name="fin")

        with tc.tile_pool(name="tp", bufs=1, space="PSUM") as TP, \
             tc.tile_pool(name="lps", bufs=2, space="PSUM") as LPS, \
             tc.tile_pool(name="pps", bufs=2, space="PSUM") as PPS, \
             tc.tile_pool(name="sm", bufs=1, space="PSUM") as SM, \
             tc.tile_pool(name="lsb", bufs=4) as LSB:

            t_rows = TP.tile([128, 256], F32, tag="t_rows", name="t_rows")

            def filler():
                w = PPS.tile([128, 2, 256], F32, tag="ps", name="warm")
                nc.tensor.matmul(w[:].rearrange("p a b -> p (a b)"),
                                 wsb[:, 0:128], wsb[:], start=True, stop=True)

            def stage_a(gi):
                j0, g = GROUPS[gi]
                mjg = 8 * (j0 + g)
                gm = g * mjg
                off = S_OFFS[gi]
                yt = LPS.tile([128, 2, 512], F32, tag="yt", name="yt")
                for hb in range(2):
                    if mjg <= 128:
                        nc.tensor.matmul(yt[:, hb, 0:gm],
                                         z1dr[0:mjg, 0, hb * 128:(hb + 1) * 128],
                                         bA[0:mjg, off:off + gm],
                                         start=True, stop=True)
                    else:
                        rhs = bA[:, off:off + 2 * gm].rearrange(
                            "p (k n) -> p k n", k=2)
                        nc.tensor.matmul(yt[:, hb, 0:gm],
                                         z1dr[:, :, hb * 128:(hb + 1) * 128],
                                         rhs, start=True, stop=True,
                                         perf_mode=DR)
                return (gi, j0, g, mjg, yt)

            def stage_b_relu(state):
                gi, j0, g, mjg, yt = state
                gm = g * mjg
                # separate tiles per half: a shared tile would put a WAW
                # dep between the two engines and serialize the relus
                ftt0 = LSB.tile([128, 512], FP8, tag="ftt0", name="ftt0")
                ftt1 = LSB.tile([128, 512], FP8, tag="ftt1", name="ftt1")
                nc.scalar.activation(ftt0[:, 0:gm], yt[:, 0, 0:gm], AF.Relu)
                nc.vector.tensor_scalar_max(ftt1[:, 0:gm], yt[:, 1, 0:gm], 0.0)
                return state + ((ftt0, ftt1),)

            def stage_c_tmv(state):
                gi, j0, g, mjg, yt, ftt = state
                first = (gi == 0)
                last = (gi == len(GROUPS) - 1)
                for q in range(g):
                    j = j0 + q
                    for hb in range(2):
                        nc.tensor.matmul(
                            t_rows[0:32, 0:mjg],
                            rczdr[:, hb, j * 32:j * 32 + 32],
                            ftt[hb][:, q * mjg:(q + 1) * mjg],
                            start=(first and q == 0 and hb == 0),
                            stop=(last and q == g - 1 and hb == 1),
                            skip_group_check=True)

            # ---------------- preamble (interleaved with g0-g2) ----------
            for _ in range(4):
                filler()

            # Z1 = z @ W1a  -> z1dr fp8 (one copy)
            zps = PPS.tile([128, 2, 256], F32, tag="ps", name="zps")
            for nb in range(2):
                nc.tensor.matmul(zps[:, nb, :], ztb[:, nb * 128:(nb + 1) * 128],
                                 w1ab[:], start=True, stop=True)
            nc.vector.tensor_copy(z1dr[:, :, :], zps[:, :, :])

            # X strip: 0.5 z z^T columns (0.5 folded into zoc host-side)
            xs = SM.tile([128, 2, 32], F32, tag="sm", name="xs")
            for ub in range(2):
                nc.tensor.matmul(xs[:, ub, :], ztb[:, ub * 128:(ub + 1) * 128],
                                 zoc[:], start=True, stop=True)
            nc.scalar.activation(fin[:, :, 32:64], xs[:, :, :], AF.Copy)

            # rbt = relu(W1a^T z^T + W1[128] bias col)
            rbtps = PPS.tile([128, 2, 256], F32, tag="ps", name="rbtps")
            for hb in range(2):
                nc.tensor.matmul(rbtps[:, hb, :], w1ab[:, hb * 128:(hb + 1) * 128],
                                 ztb[:], start=True, stop=True)
            for hb in range(2):
                nc.scalar.activation(rbt[:, hb, :], rbtps[:, hb, :], AF.Relu,
                                     bias=w1b[:, hb:hb + 1])

            st0 = stage_a(0)

            # Q^T = W2^T relu(ZB)^T
            qtps = PPS.tile([128, 2, 256], F32, tag="ps", name="qtps")
            for hb in range(2):
                nc.tensor.matmul(qtps[:, 0, :], w2h[hb][:], rbt[:, hb, :],
                                 start=(hb == 0), stop=(hb == 1))
            nc.vector.tensor_copy(qtb[:], qtps[:, 0, :])
            nc.vector.tensor_mul(sqb[:], qtb[:], qtb[:])

            filler()
            st1 = stage_a(1)

            # R = Q @ W2^T  -> rsb
            rsps = [PPS.tile([128, 2, 256], F32, tag="ps", name=f"rsps{nb}")
                    for nb in range(2)]
            for nb in range(2):
                nc.tensor.matmul(rsps[nb][:, 0, :],
                                 qtb[:, nb * 128:(nb + 1) * 128], w2tb[:],
                                 start=True, stop=True)
                nc.vector.tensor_copy(rsb[nb][:], rsps[nb][:, 0, :])

            st2 = stage_a(2)
            stage_c_tmv(stage_b_relu(st0))

            # rc[h, j] = R[i_j, h] via one-hot select
            rcps = SM.tile([128, 2, 32], F32, tag="sm", name="rcps")
            for hb in range(2):
                for nb in range(2):
                    nc.tensor.matmul(rcps[:, hb, :],
                                     rsb[nb][:, hb * 128:(hb + 1) * 128],
                                     ocb[nb][:],
                                     start=(nb == 0), stop=(nb == 1))
            nc.vector.tensor_copy(rc_sb[:, :, :], rcps[:, :, :])
            for hb in range(2):
                dst = rczdr[:, hb, :].rearrange("p (j k) -> p j k", k=33)[:, :, 0:1]
                nc.vector.tensor_copy(dst, rc_sb[:, hb, :].unsqueeze(2))

            # qq[n] = |q_n|^2 ; dg = 0.5 * onehot * tanh(qq)
            qq = SM.tile([128, 2, 32], F32, tag="sm", name="qq")
            for nb in range(2):
                nc.tensor.matmul(qq[:, nb, 0:1], sqb[:, nb * 128:(nb + 1) * 128],
                                 onescol[:], start=True, stop=True)
            nc.scalar.activation(tqh[:].rearrange("p (u j) -> p u j", u=2),
                                 qq[:, :, 0:1], AF.Tanh)
            for ib in range(2):
                nc.vector.tensor_scalar(dg[:, ib, :], ocb[ib][:], tqh[:, ib:ib + 1],
                                        0.5, ALU.mult, ALU.mult)

            # x-strip half of the output ships as soon as it exists
            nc.sync.dma_start(out_d[:].rearrange("p (u j) -> p u j", u=2)[:, :, 32:64],
                              fin[:, :, 32:64])

            # ---------------- remaining row loop ----------------
            pend = [st1, st2]
            for gi in range(3, len(GROUPS)):
                st = stage_a(gi)
                pend.append(st)
                stage_c_tmv(stage_b_relu(pend.pop(0)))
            for st in pend:
                stage_c_tmv(stage_b_relu(st))

            # ---------------- tail ----------------
            nc.scalar.activation(spT[:], t_rows[0:32, :], AF.Copy)
            pst2 = SM.tile([128, 2, 32], BF16, tag="sm", name="pst2")
            for vb in range(2):
                nc.tensor.transpose(pst2[:, vb, :], spT[:, vb * 128:(vb + 1) * 128],
                                    ident32)
            nc.vector.tensor_copy(spc[:, :, :], pst2[:, :, :])
            up = SM.tile([128, 2, 32], F32, tag="sm", name="up")
            for ub in range(2):
                nc.tensor.matmul(up[:, ub, :],
                                 ahbdr[:, :, ub * 128:(ub + 1) * 128],
                                 spc[:, :, :],
                                 start=True, stop=True, perf_mode=DR)
            nc.vector.tensor_mul(wt[:], up[:, :, :].rearrange("p u j -> p (u j)"),
                                 degf)
            nc.scalar.activation(tht[:], wt[:], AF.Tanh)
            nc.vector.scalar_tensor_tensor(
                fin[:, :, 0:32],
                tht[:].rearrange("p (u j) -> p u j", u=2),
                0.5, dg[:, :, :], ALU.mult, ALU.add)
        nc.sync.dma_start(out_d[:].rearrange("p (u j) -> p u j", u=2)[:, :, 0:32],
                          fin[:, :, 0:32])

    nc.finalize()
    return nc


def _get_program():
    global _PROGRAM
    if _PROGRAM is None:
        _PROGRAM = _build_program()
    return _PROGRAM


def kernel(z, adj, W1, W2):
    global LAST_RESULTS
    from concourse.bass_utils import run_bass_kernel_spmd
    import concourse.mybir as _mybir
    import ml_dtypes

    bf = ml_dtypes.bfloat16
    f8 = _mybir.dt.np(_mybir.dt.float8e4)
    z = np.asarray(z, np.float32)
    adj = np.asarray(adj, np.float32)
    W1 = np.asarray(W1, np.float32)
    W2 = np.asarray(W2, np.float32)

    idx = np.arange(N)
    Ahat = np.maximum(adj, np.eye(N, dtype=np.float32))
    zt = z.T  # [128, 256]
    CSex = np.vstack([np.zeros((1, N), np.float32), np.cumsum(Ahat, axis=0)[:-1]])

    nc = _get_program()
    in_maps = []
    for c in range(NCORES):
        ii = np.arange(c, N, NCORES)
        OC = np.zeros((N, NPC), np.float32)
        OC[ii, np.arange(NPC)] = 1.0

        with np.errstate(divide="ignore"):
            degM = np.where(idx[:, None] < ii[None, :],
                            CSex[ii].T ** -0.5, 0.0).astype(np.float32)  # [w, j]

        bA = np.zeros((128, BA_COLS), f8)
        for gi, (j0, g) in enumerate(GROUPS):
            mjg = 8 * (j0 + g)
            gm = g * mjg
            off = S_OFFS[gi]
            nk = 1 if mjg <= 128 else 2
            for ko in range(nk):
                w0 = ko * 128
                sz = min(mjg - w0, 128)
                # deg_j(v) * Ahat[v, u] * deg_j(u)^2   (both masks applied)
                blk = (Ahat[w0:w0 + sz, None, 0:mjg]
                       * degM[w0:w0 + sz, j0:j0 + g, None]
                       * (degM[None, 0:mjg, j0:j0 + g] ** 2).transpose(0, 2, 1))
                bA[0:sz, off + ko * gm:off + ko * gm + gm] = \
                    blk.reshape(sz, gm).astype(f8)
        for ko in range(2):
            bA[:, S_COLS + ko * 256:S_COLS + (ko + 1) * 256] = \
                Ahat[ko * 128:(ko + 1) * 128].astype(f8)

        bB = np.zeros((128, BB_COLS), bf)
        bB[:, ZT:ZT + 256] = zt.astype(bf)
        bB[:, W1A:W1A + 256] = W1[0:128].astype(bf)
        bB[:, W2H:W2H + 128] = W2[0:128].astype(bf)
        bB[:, W2H + 128:W2H + 256] = W2[128:256].astype(bf)
        bB[:, W2T:W2T + 256] = W2.T.astype(bf)
        bB[:, ZOC:ZOC + 32] = (0.5 * zt[:, ii]).astype(bf)
        bB[:, W1B] = W1[128, 0:128].astype(bf)
        bB[:, W1B + 1] = W1[128, 128:256].astype(bf)

        bC = np.zeros((128, BC_COLS), bf)
        degf = np.zeros((128, 64), np.float32)
        for ub in range(2):
            degf[:, ub * 32:(ub + 1) * 32] = degM[ub * 128:(ub + 1) * 128, :]
        bC[:, DEGF:DEGF + 128] = degf.view(bf)
        bC[:, OCB:OCB + 32] = OC[0:128].astype(bf)
        bC[:, OCB + 32:OCB + 64] = OC[128:256].astype(bf)
        bC[0:32, IDT:IDT + 32] = np.eye(32, dtype=np.float32).astype(bf)
        in_maps.append({"bA": bA, "bB": bB, "bC": bC})

    res = run_bass_kernel_spmd(nc, in_maps, list(range(NCORES)),
                               trace=TRACE, **TRACE_KW)
    LAST_RESULTS = res

    supp = np.zeros((N, N), np.float32)
    x = np.zeros((N, N), np.float32)
    for c in range(NCORES):
        ii = np.arange(c, N, NCORES)
        out_r = np.asarray(res.results[c]["outp"], np.float32).reshape(128, 2, 64)
        supp[ii, :] = out_r[:, :, 0:32].transpose(2, 1, 0).reshape(NPC, N)
        x[:, ii] = out_r[:, :, 32:64].transpose(1, 0, 2).reshape(N, NPC)
    return (x + supp + supp.T).astype(np.float32)


# revision 6
# speedup vs baseline: 1.1781x; 1.1781x over previous
"""Trainium2 Bass kernel for nn_AutoregressiveDecoder (gnn_message_passing).

Math (derived from the reference):
  With Ahat = max(adj, I), CS[i,u] = sum_{v<i} Ahat[v,u], deg_i = CS[i]^-1/2,
  row i of supp reduces to
    supp[i,u<i] = 0.5*tanh(deg_i(u) * (Ahat @ (deg_i^2 * relu(Yt_i)^T r_i))[u])
    supp[i,i]   = 0.5*tanh(q_i . q_i)
  where Yt_i = Z1^T (D_i Ahat), Z1 = z @ W1[:128],
    q_i = relu(Z1[i] + W1[128]) @ W2,  r_i = W2 @ q_i.
  Output = 0.5 z z^T + supp + supp^T.

v2 layout (vs baseline): deg is a pure function of adj, so ALL deg math is
host-precomputed: deg_j(v) AND deg_j(u)^2 are folded into the fp8 stage_a
operand (b3'), and the outer deg_i(u) ships as an f32 table (degf). This
deletes the on-device CS/rsqrt/transpose chain that serialized the old
preamble. W1's helper-row bias rides as an ACT per-partition bias column.
Inputs ship as 3 dram tensors in consumption order; PE filler matmuls keep
HAM activity up so the row loop runs warm.
"""

import numpy as np

N = 256
NCORES = 8
NPC = N // NCORES  # 32 rows per core

# (start_row, g) per group; mjg = 8*(start+g)
GROUPS = [(0, 8), (8, 4), (12, 4)] + [(j, 2) for j in range(16, 32, 2)]


def _s_offsets():
    offs = {}
    off = 0
    for gi, (j0, g) in enumerate(GROUPS):
        mjg = 8 * (j0 + g)
        nk = 1 if mjg <= 128 else 2
        offs[gi] = off
        off += nk * g * mjg
    return offs, off


S_OFFS, S_COLS = _s_offsets()  # S_COLS == 7808
BA_COLS = S_COLS + 512        # | b3' | ahbdr(2x256 DR) |

# bB (bf16) column map
ZT, W1A, W2H, W2T, ZOC, W1B = 0, 256, 512, 768, 1024, 1056
BB_COLS = 1058
# bC (bf16) column map: degf f32-bytes, one-hots, 32-identity
DEGF, OCB, IDT = 0, 128, 192
BC_COLS = 224

_PROGRAM = None
LAST_RESULTS = None
TRACE = False
TRACE_KW = {}


def _build_program():
    import concourse.bacc as bacc
    import concourse.mybir as mybir
    from concourse import tile

    F32 = mybir.dt.float32
    BF16 = mybir.dt.bfloat16
    FP8 = mybir.dt.float8e4
    AF = mybir.ActivationFunctionType
    ALU = mybir.AluOpType
    DR = mybir.MatmulPerfMode.DoubleRow

    nc = bacc.Bacc()

    bA_d = nc.dram_tensor("bA", [128, BA_COLS], FP8, kind="ExternalInput")
    bB_d = nc.dram_tensor("bB", [128, BB_COLS], BF16, kind="ExternalInput")
    bC_d = nc.dram_tensor("bC", [128, BC_COLS], BF16, kind="ExternalInput")
    out_d = nc.dram_tensor("outp", [128, 128], F32, kind="ExternalOutput")

    with tile.TileContext(nc) as tc, tc.tile_pool(name="persist", bufs=1) as P:
        bA = P.tile([128, BA_COLS], FP8, tag="bA", name="bA")
        bB = P.tile([128, BB_COLS], BF16, tag="bB", name="bB")
        bC = P.tile([128, BC_COLS], BF16, tag="bC", name="bC")
        # priority order: bB gates the preamble, bA chunks stream in
        # consumption order, bC is tail-only
        nc.sync.dma_start(bB[:], bB_d[:])
        nc.sync.dma_start(bC[:], bC_d[:])
        nc.sync.dma_start(bA[:, 0:1408], bA_d[:, 0:1408])
        nc.sync.dma_start(bA[:, 1408:4224], bA_d[:, 1408:4224])
        nc.sync.dma_start(bA[:, 4224:BA_COLS], bA_d[:, 4224:BA_COLS])

        ztb = bB[:, ZT:ZT + 256]
        w1ab = bB[:, W1A:W1A + 256]
        w2h = [bB[:, W2H:W2H + 128], bB[:, W2H + 128:W2H + 256]]
        w2tb = bB[:, W2T:W2T + 256]
        zoc = bB[:, ZOC:ZOC + 32]                   # 0.5-folded z cols
        w1b = bB[:, W1B:W1B + 2]                    # W1[128] as 2 bias cols
        ahbdr = bA[:, S_COLS:S_COLS + 512].rearrange("p (k c) -> p k c", k=2)
        degf = bC[:, DEGF:DEGF + 128].bitcast(F32)  # [128, 64]
        ocb = [bC[:, OCB:OCB + 32], bC[:, OCB + 32:OCB + 64]]
        ident32 = bC[0:32, IDT:IDT + 32]

        wsb = P.tile([128, 512], BF16, tag="wsb", name="wsb")
        nc.vector.memset(wsb[:], 0.0)
        onescol = P.tile([128, 1], BF16, tag="onescol", name="onescol")
        nc.gpsimd.memset(onescol[:], 1.0)
        rczdr = P.tile([128, 2, 1056], FP8, tag="rczdr", name="rczdr")
        nc.gpsimd.memset(rczdr[:], 0.0)

        # persistent SBUF intermediates
        z1dr = P.tile([128, 2, 256], FP8, tag="z1dr", name="z1dr")
        rbt = P.tile([128, 2, 256], BF16, tag="rbt", name="rbt")
        qtb = P.tile([128, 256], BF16, tag="qtb", name="qtb")
        sqb = P.tile([128, 256], BF16, tag="sqb", name="sqb")
        rsb = [P.tile([128, 256], BF16, tag=f"rsb{nb}", name=f"rsb{nb}")
               for nb in range(2)]
        rc_sb = P.tile([128, 2, 32], BF16, tag="rc_sb", name="rc_sb")
        tqh = P.tile([128, 2], F32, tag="tqh", name="tqh")
        dg = P.tile([128, 2, 32], F32, tag="dg", name="dg")
        spT = P.tile([32, 256], BF16, tag="spT", name="spT")
        spc = P.tile([128, 2, 32], FP8, tag="spc", name="spc")
        wt = P.tile([128, 64], F32, tag="wt", name="wt")
        tht = P.tile([128, 64], F32, tag="tht", name="tht")
        fin = P.tile([128, 2, 64], F32, tag="fin", name="fin")

        with tc.tile_pool(name="tp", bufs=1, space="PSUM") as TP, \
             tc.tile_pool(name="lps", bufs=2, space="PSUM") as LPS, \
             tc.tile_pool(name="pps", bufs=2, space="PSUM") as PPS, \
             tc.tile_pool(name="sm", bufs=1, space="PSUM") as SM, \
             tc.tile_pool(name="lsb", bufs=4) as LSB:

            t_rows = TP.tile([128, 256], F32, tag="t_rows", name="t_rows")

            def filler():
                w = PPS.tile([128, 2, 256], F32, tag="ps", name="warm")
                nc.tensor.matmul(w[:].rearrange("p a b -> p (a b)"),
                                 wsb[:, 0:128], wsb[:], start=True, stop=True)

            def stage_a(gi):
                j0, g = GROUPS[gi]
                mjg = 8 * (j0 + g)
                gm = g * mjg
                off = S_OFFS[gi]
                yt = LPS.tile([128, 2, 512], F32, tag="yt", name="yt")
                for hb in range(2):
                    if mjg <= 128:
                        nc.tensor.matmul(yt[:, hb, 0:gm],
                                         z1dr[0:mjg, 0, hb * 128:(hb + 1) * 128],
                                         bA[0:mjg, off:off + gm],
                                         start=True, stop=True)
                    else:
                        rhs = bA[:, off:off + 2 * gm].rearrange(
                            "p (k n) -> p k n", k=2)
                        nc.tensor.matmul(yt[:, hb, 0:gm],
                                         z1dr[:, :, hb * 128:(hb + 1) * 128],
                                         rhs, start=True, stop=True,
                                         perf_mode=DR)
                return (gi, j0, g, mjg, yt)

            def stage_b_relu(state):
                gi, j0, g, mjg, yt = state
                gm = g * mjg
                # separate tiles per half: a shared tile would put a WAW
                # dep between the two engines and serialize the relus
                ftt0 = LSB.tile([128, 512], FP8, tag="ftt0", name="ftt0")
                ftt1 = LSB.tile([128, 512], FP8, tag="ftt1", name="ftt1")
                nc.scalar.activation(ftt0[:, 0:gm], yt[:, 0, 0:gm], AF.Relu)
                nc.vector.tensor_scalar_max(ftt1[:, 0:gm], yt[:, 1, 0:gm], 0.0)
                return state + ((ftt0, ftt1),)

            def stage_c_tmv(state):
                gi, j0, g, mjg, yt, ftt = state
                first = (gi == 0)
                last = (gi == len(GROUPS) - 1)
                for q in range(g):
                    j = j0 + q
                    for hb in range(2):
                        nc.tensor.matmul(
                            t_rows[0:32, 0:mjg],
                            rczdr[:, hb, j * 32:j * 32 + 32],
                            ftt[hb][:, q * mjg:(q + 1) * mjg],
                            start=(first and q == 0 and hb == 0),
                            stop=(last and q == g - 1 and hb == 1),
                            skip_group_check=True)

            # ---------------- preamble (interleaved with g0-g2) ----------
            for _ in range(4):
                filler()

            # Z1 = z @ W1a  -> z1dr fp8 (one copy)
            zps = PPS.tile([128, 2, 256], F32, tag="ps", name="zps")
            for nb in range(2):
                nc.tensor.matmul(zps[:, nb, :], ztb[:, nb * 128:(nb + 1) * 128],
                                 w1ab[:], start=True, stop=True)
            nc.vector.tensor_copy(z1dr[:, :, :], zps[:, :, :])

            # X strip: 0.5 z z^T columns (0.5 folded into zoc host-side)
            xs = SM.tile([128, 2, 32], F32, tag="sm", name="xs")
            for ub in range(2):
                nc.tensor.matmul(xs[:, ub, :], ztb[:, ub * 128:(ub + 1) * 128],
                                 zoc[:], start=True, stop=True)
            nc.scalar.activation(fin[:, :, 32:64], xs[:, :, :], AF.Copy)

            # rbt = relu(W1a^T z^T + W1[128] bias col)
            rbtps = PPS.tile([128, 2, 256], F32, tag="ps", name="rbtps")
            for hb in range(2):
                nc.tensor.matmul(rbtps[:, hb, :], w1ab[:, hb * 128:(hb + 1) * 128],
                                 ztb[:], start=True, stop=True)
            for hb in range(2):
                nc.scalar.activation(rbt[:, hb, :], rbtps[:, hb, :], AF.Relu,
                                     bias=w1b[:, hb:hb + 1])

            st0 = stage_a(0)

            # Q^T = W2^T relu(ZB)^T
            qtps = PPS.tile([128, 2, 256], F32, tag="ps", name="qtps")
            for hb in range(2):
                nc.tensor.matmul(qtps[:, 0, :], w2h[hb][:], rbt[:, hb, :],
                                 start=(hb == 0), stop=(hb == 1))
            nc.vector.tensor_copy(qtb[:], qtps[:, 0, :])
            nc.vector.tensor_mul(sqb[:], qtb[:], qtb[:])

            filler()
            st1 = stage_a(1)

            # R = Q @ W2^T  -> rsb
            rsps = [PPS.tile([128, 2, 256], F32, tag="ps", name=f"rsps{nb}")
                    for nb in range(2)]
            for nb in range(2):
                nc.tensor.matmul(rsps[nb][:, 0, :],
                                 qtb[:, nb * 128:(nb + 1) * 128], w2tb[:],
                                 start=True, stop=True)
                nc.vector.tensor_copy(rsb[nb][:], rsps[nb][:, 0, :])

            st2 = stage_a(2)
            stage_c_tmv(stage_b_relu(st0))

            # rc[h, j] = R[i_j, h] via one-hot select
            rcps = SM.tile([128, 2, 32], F32, tag="sm", name="rcps")
            for hb in range(2):
                for nb in range(2):
                    nc.tensor.matmul(rcps[:, hb, :],
                                     rsb[nb][:, hb * 128:(hb + 1) * 128],
                                     ocb[nb][:],
                                     start=(nb == 0), stop=(nb == 1))
            nc.vector.tensor_copy(rc_sb[:, :, :], rcps[:, :, :])
            for hb in range(2):
                dst = rczdr[:, hb, :].rearrange("p (j k) -> p j k", k=33)[:, :, 0:1]
                nc.vector.tensor_copy(dst, rc_sb[:, hb, :].unsqueeze(2))

            # qq[n] = |q_n|^2 ; dg = 0.5 * onehot * tanh(qq)
            qq = SM.tile([128, 2, 32], F32, tag="sm", name="qq")
            for nb in range(2):
                nc.tensor.matmul(qq[:, nb, 0:1], sqb[:, nb * 128:(nb + 1) * 128],
                                 onescol[:], start=True, stop=True)
            nc.scalar.activation(tqh[:].rearrange("p (u j) -> p u j", u=2),
                                 qq[:, :, 0:1], AF.Tanh)
            for ib in range(2):
                nc.vector.tensor_scalar(dg[:, ib, :], ocb[ib][:], tqh[:, ib:ib + 1],
                                        0.5, ALU.mult, ALU.mult)

            # x-strip half of the output ships as soon as it exists
            nc.sync.dma_start(out_d[:].rearrange("p (u j) -> p u j", u=2)[:, :, 32:64],
                              fin[:, :, 32:64])

            # ---------------- remaining row loop ----------------
            pend = [st1, st2]
            for gi in range(3, len(GROUPS)):
                st = stage_a(gi)
                pend.append(st)
                stage_c_tmv(stage_b_relu(pend.pop(0)))
            for st in pend:
                stage_c_tmv(stage_b_relu(st))

            # ---------------- tail ----------------
            nc.scalar.activation(spT[:], t_rows[0:32, :], AF.Copy)
            pst2 = SM.tile([128, 2, 32], BF16, tag="sm", name="pst2")
            for vb in range(2):
                nc.tensor.transpose(pst2[:, vb, :], spT[:, vb * 128:(vb + 1) * 128],
                                    ident32)
            nc.vector.tensor_copy(spc[:, :, :], pst2[:, :, :])
            up = SM.tile([128, 2, 32], F32, tag="sm", name="up")
            for ub in range(2):
                nc.tensor.matmul(up[:, ub, :],
                                 ahbdr[:, :, ub * 128:(ub + 1) * 128],
                                 spc[:, :, :],
                                 start=True, stop=True, perf_mode=DR)
            nc.vector.tensor_mul(wt[:], up[:, :, :].rearrange("p u j -> p (u j)"),
                                 degf)
            nc.scalar.activation(tht[:], wt[:], AF.Tanh)
            nc.vector.scalar_tensor_tensor(
                fin[:, :, 0:32],
                tht[:].rearrange("p (u j) -> p u j", u=2),
                0.5, dg[:, :, :], ALU.mult, ALU.add)
        nc.sync.dma_start(out_d[:].rearrange("p (u j) -> p u j", u=2)[:, :, 0:32],
                          fin[:, :, 0:32])

    nc.finalize()
    return nc


def _get_program():
    global _PROGRAM
    if _PROGRAM is None:
        _PROGRAM = _build_program()
    return _PROGRAM


def kernel(z, adj, W1, W2):
    global LAST_RESULTS
    from concourse.bass_utils import run_bass_kernel_spmd
    import concourse.mybir as _mybir
    import ml_dtypes

    bf = ml_dtypes.bfloat16
    f8 = _mybir.dt.np(_mybir.dt.float8e4)
    z = np.asarray(z, np.float32)
    adj = np.asarray(adj, np.float32)
    W1 = np.asarray(W1, np.float32)
    W2 = np.asarray(W2, np.float32)

    idx = np.arange(N)
    Ahat = np.maximum(adj, np.eye(N, dtype=np.float32))
    zt = z.T  # [128, 256]
    CSex = np.vstack([np.zeros((1, N), np.float32), np.cumsum(Ahat, axis=0)[:-1]])

    nc = _get_program()
    in_maps = []
    for c in range(NCORES):
        ii = np.arange(c, N, NCORES)
        OC = np.zeros((N, NPC), np.float32)
        OC[ii, np.arange(NPC)] = 1.0

        with np.errstate(divide="ignore"):
            degM = np.where(idx[:, None] < ii[None, :],
                            CSex[ii].T ** -0.5, 0.0).astype(np.float32)  # [w, j]

        bA = np.zeros((128, BA_COLS), f8)
        for gi, (j0, g) in enumerate(GROUPS):
            mjg = 8 * (j0 + g)
            gm = g * mjg
            off = S_OFFS[gi]
            nk = 1 if mjg <= 128 else 2
            for ko in range(nk):
                w0 = ko * 128
                sz = min(mjg - w0, 128)
                # deg_j(v) * Ahat[v, u] * deg_j(u)^2   (both masks applied)
                blk = (Ahat[w0:w0 + sz, None, 0:mjg]
                       * degM[w0:w0 + sz, j0:j0 + g, None]
                       * (degM[None, 0:mjg, j0:j0 + g] ** 2).transpose(0, 2, 1))
                bA[0:sz, off + ko * gm:off + ko * gm + gm] = \
                    blk.reshape(sz, gm).astype(f8)
        for ko in range(2):
            bA[:, S_COLS + ko * 256:S_COLS + (ko + 1) * 256] = \
                Ahat[ko * 128:(ko + 1) * 128].astype(f8)

        bB = np.zeros((128, BB_COLS), bf)
        bB[:, ZT:ZT + 256] = zt.astype(bf)
        bB[:, W1A:W1A + 256] = W1[0:128].astype(bf)
        bB[:, W2H:W2H + 128] = W2[0:128].astype(bf)
        bB[:, W2H + 128:W2H + 256] = W2[128:256].astype(bf)
        bB[:, W2T:W2T + 256] = W2.T.astype(bf)
        bB[:, ZOC:ZOC + 32] = (0.5 * zt[:, ii]).astype(bf)
        bB[:, W1B] = W1[128, 0:128].astype(bf)
        bB[:, W1B + 1] = W1[128, 128:256].astype(bf)

        bC = np.zeros((128, BC_COLS), bf)
        degf = np.zeros((128, 64), np.float32)
        for ub in range(2):
            degf[:, ub * 32:(ub + 1) * 32] = degM[ub * 128:(ub + 1) * 128, :]
        bC[:, DEGF:DEGF + 128] = degf.view(bf)
        bC[:, OCB:OCB + 32] = OC[0:128].astype(bf)
        bC[:, OCB + 32:OCB + 64] = OC[128:256].astype(bf)
        bC[0:32, IDT:IDT + 32] = np.eye(32, dtype=np.float32).astype(bf)
        in_maps.append({"bA": bA, "bB": bB, "bC": bC})

    res = run_bass_kernel_spmd(nc, in_maps, list(range(NCORES)),
                               trace=TRACE, **TRACE_KW)
    LAST_RESULTS = res

    supp = np.zeros((N, N), np.float32)
    x = np.zeros((N, N), np.float32)
    for c in range(NCORES):
        ii = np.arange(c, N, NCORES)
        out_r = np.asarray(res.results[c]["outp"], np.float32).reshape(128, 2, 64)
        supp[ii, :] = out_r[:, :, 0:32].transpose(2, 1, 0).reshape(NPC, N)
        x[:, ii] = out_r[:, :, 32:64].transpose(1, 0, 2).reshape(N, NPC)
    return (x + supp + supp.T).astype(np.float32)
